# revision 2
# baseline (speedup 1.0000x reference)
"""Trainium2 Bass kernel for the ODEFunc GNN message-passing module (final).

v4 + trace-driven fixes:
- real DMA transfer time gates the start: inputs split over 3 hardware
  queues (sync/scalar/gpsimd) in need-order
- per-group S PSUM tiles (P/num dependencies were whole-tile in v4, so the
  combination couldn't overlap the later S groups); the group tiles reuse
  the banks of kT/scT/yj which are dead by then
- S node-groups (2,2,1,1): the first group only needs tanh nodes 0-1, so
  PE and DVE overlap the scalar tanh chain instead of trailing it
- barycentric denominator back on DVE (gpsimd chain was slower and the
  static schedule head-of-line blocked the DVE tail behind it)
- kT bias-add on DVE, scalar runs only exp + tanh chain + epilogue
- bf16 U/identity/transpose
"""

import ml_dtypes
import numpy as np

B, N, H, O = 2, 512, 128, 32
NC = 8
CPB = NC // B
IPC = N // CPB
NCHUNK = N // 128

M = 6
XRANGE = 2.0

_CACHE = {}
LAST_RESULTS = None


def _nodes():
    m = np.arange(M)
    xm = (XRANGE * np.cos(np.pi * m / (M - 1))).astype(np.float32)
    w = (-1.0) ** m
    w[0] *= 0.5
    w[-1] *= 0.5
    return xm, w.astype(np.float32)


def _build():
    from contextlib import ExitStack

    import concourse.tile as tile
    from concourse import bacc, mybir

    f32 = mybir.dt.float32
    bf16 = mybir.dt.bfloat16
    AF = mybir.ActivationFunctionType
    ALU = mybir.AluOpType

    nc = bacc.Bacc(trn_type="TRN2")

    # zW  [128, 513] bf16: zTi | WqTs | W1jT | W1iT | onescol
    # zC  [128, 512] bf16: zT
    # pkF [128, 17]  f32 : bqs bk b2 b3 b4 | xnodes | wts
    # pk32 [32, 896] bf16: WkT | sT | (row0: b1row | onesrow)
    # maskT [128, 512] bf16 ; pkC [128, 512] bf16: W2T | W3T | W4T | ident
    zW = nc.dram_tensor("zW", [128, 513], bf16, kind="ExternalInput")
    zC = nc.dram_tensor("zC", [H, N], bf16, kind="ExternalInput")
    pkF = nc.dram_tensor("pkF", [128, 5 + 2 * M], f32, kind="ExternalInput")
    pk32 = nc.dram_tensor("pk32", [O, 896], bf16, kind="ExternalInput")
    Madd = nc.dram_tensor("Madd", [128, N], bf16, kind="ExternalInput")
    pkC = nc.dram_tensor("pkC", [128, 384], bf16, kind="ExternalInput")
    out = nc.dram_tensor("out", [H, IPC], f32, kind="ExternalOutput")

    with tile.TileContext(nc) as tc, ExitStack() as ctx:
        const = ctx.enter_context(tc.tile_pool(name="const", bufs=1))
        work = ctx.enter_context(tc.tile_pool(name="work", bufs=2))
        pbig = ctx.enter_context(tc.tile_pool(name="pbig", bufs=1, space="PSUM"))
        small = ctx.enter_context(tc.tile_pool(name="small", bufs=1, space="PSUM"))

        zW_t = const.tile([128, 513], bf16, tag="zW", name="zW_sb")
        zC_t = const.tile([H, N], bf16, tag="zC", name="zC_sb")
        pkF_t = const.tile([128, 5 + 2 * M], f32, tag="pkF", name="pkF_sb")
        pk32_t = const.tile([O, 896], bf16, tag="pk32", name="pk32_sb")
        Madd_t = const.tile([128, N], bf16, tag="Madd", name="Madd_sb")
        pkC_t = const.tile([128, 384], bf16, tag="pkC", name="pkC_sb")

        nc.sync.dma_start(zC_t[:], zC[:, :])
        nc.scalar.dma_start(pk32_t[:], pk32[:, :])
        nc.scalar.dma_start(zW_t[:], zW[:, :])
        nc.sync.dma_start(pkF_t[:], pkF[:, :])
        nc.gpsimd.dma_start(pkC_t[:], pkC[:, :])
        nc.gpsimd.dma_start(Madd_t[:], Madd[:, :])

        zTi = zW_t[:, 0:128]
        WqTs = zW_t[:, 128:256]
        W1jT = zW_t[:, 256:384]
        W1iT = zW_t[:, 384:512]
        zT = zC_t[:, :]
        WkT = pk32_t[:, 0:128]
        sT = pk32_t[:, 128:640]
        b1row = pk32_t[0:1, 640:768]
        onesrow = pk32_t[0:1, 768:896]
        bqs = pkF_t[:, 0:1]
        bk = pkF_t[:, 1:2]
        b32 = pkF_t[:, 2:3]
        b4 = pkF_t[:, 4:5]
        xnodes = pkF_t[:, 5 : 5 + M]
        wts = pkF_t[:, 5 + M : 5 + 2 * M]
        W32T = pkC_t[:, 0:128]
        W4T = pkC_t[:, 128:256]
        ident = pkC_t[:, 256:384]

        # ---- PE head ----
        kT_ps = pbig.tile([H, N], f32, tag="kT", name="kT_ps")
        nc.tensor.matmul(kT_ps[:], WkT, sT, start=True, stop=True)
        qs_ps = small.tile([H, IPC], f32, tag="sm1", name="qs_ps")
        nc.tensor.matmul(qs_ps[:], WqTs, zTi, start=True, stop=True)
        xi_ps = small.tile([IPC, H], f32, tag="sm2", name="xi_ps")
        nc.tensor.matmul(xi_ps[:], zTi, W1iT, start=True, stop=False)
        nc.tensor.matmul(xi_ps[:], onesrow, b1row, start=False, stop=True)
        yj_ps = pbig.tile([128, NCHUNK, H], f32, tag="yj", name="yj_ps")
        for c in range(NCHUNK):
            nc.tensor.matmul(
                yj_ps[:, c, :],
                zT[:, 128 * c : 128 * (c + 1)],
                W1jT,
                start=True,
                stop=True,
            )

        # DVE: bias-adds for q and k
        qsT_t = work.tile([H, IPC], bf16, tag="qsT", name="qsT_sb")
        nc.vector.tensor_scalar(qsT_t[:], qs_ps[:], bqs, None, ALU.add)
        kT_t = work.tile([H, N], bf16, tag="kT", name="kT_sb")
        nc.vector.tensor_scalar(kT_t[:], kT_ps[:], bk, None, ALU.add)

        scT_ps = pbig.tile([128, N], f32, tag="scT", name="scT_ps")
        for c in range(NCHUNK):
            nc.tensor.matmul(
                scT_ps[:, 128 * c : 128 * (c + 1)],
                ident,
                Madd_t[:, 128 * c : 128 * (c + 1)],
                start=True,
                stop=False,
            )
        for c in range(NCHUNK):
            nc.tensor.matmul(
                scT_ps[:, 128 * c : 128 * (c + 1)],
                kT_t[:, 128 * c : 128 * (c + 1)],
                qsT_t[:],
                start=False,
                stop=True,
            )

        # scalar: tanh node chain with exp slotted after T1
        T_all = const.tile([128, NCHUNK, M * 128 + 1], bf16, tag="T_all", name="T_all")
        nc.gpsimd.memset(T_all[:, :, M * 128 : M * 128 + 1], 1.0)

        def t_act(m):
            nc.scalar.activation(
                T_all[:, :, 128 * m : 128 * (m + 1)],
                yj_ps[:, :, :],
                AF.Tanh,
                bias=xnodes[:, m : m + 1],
            )

        with tc.high_priority():
            t_act(0)
            t_act(1)
        eT_t = work.tile([128, N], bf16, tag="eT", name="eT_sb")
        nc.scalar.activation(eT_t[:], scT_ps[:], AF.Exp)
        for m in range(2, M):
            t_act(m)

        # ---- DVE: D, R0, mask, R, den, rden ----
        D_t = work.tile([IPC, H, M], f32, tag="D", name="D_t")
        xi_b = xi_ps[:, :].unsqueeze(2).broadcast_to((IPC, H, M))
        xn_b = xnodes.unsqueeze(1).broadcast_to((IPC, H, M))
        wt_b = wts.unsqueeze(1).broadcast_to((IPC, H, M))
        nc.vector.tensor_sub(D_t[:, :, :], xi_b, xn_b)
        R0_t = work.tile([IPC, H, M], f32, tag="R0", name="R0_t")
        nc.vector.reciprocal_approx_fast(R0_t[:, :, :], D_t[:, :, :])
        R_t = work.tile([IPC, H, M], f32, tag="R", name="R_t")
        nc.vector.tensor_mul(R_t[:, :, :], R0_t[:, :, :], wt_b)
        den_t = work.tile([IPC, H], f32, tag="den", name="den_t")
        nc.vector.tensor_reduce(den_t[:], R_t[:, :, :], mybir.AxisListType.X, ALU.add)
        rden_t = work.tile([IPC, H], f32, tag="rden", name="rden_t")
        nc.vector.reciprocal_approx_fast(rden_t[:], den_t[:])

        # ---- S matmul groups (2,2,1,1+ones) into recycled PSUM banks ----
        def s_group(pool, tag, tcol0, width, name):
            sp = pool.tile([IPC, width], f32, tag=tag, name=name)
            for c in range(NCHUNK):
                nc.tensor.matmul(
                    sp[:, :],
                    eT_t[:, 128 * c : 128 * (c + 1)],
                    T_all[:, c, tcol0 : tcol0 + width],
                    start=(c == 0),
                    stop=(c == NCHUNK - 1),
                )
            return sp

        Sones = s_group(small, "sm3", M * 128, 1, "Sones_ps")  # softmax denom
        rs_t = work.tile([IPC, 1], f32, tag="rs", name="rs_t")
        nc.vector.reciprocal_approx_fast(rs_t[:], Sones[:, 0:1])
        S0a = s_group(pbig, "kT", 0, 256, "S0a_ps")     # nodes 0-1
        S0b = s_group(pbig, "scT", 256, 256, "S0b_ps")  # nodes 2-3
        S1 = s_group(pbig, "yj", 512, 128, "S1_ps")     # node 4
        S2 = s_group(small, "sm1", 640, 128, "S2_ps")   # node 5

        # ---- DVE tail: per-group P/num, combine, U ----
        def pmul2(S, lo, name):
            P = work.tile([IPC, H, 2], f32, tag=name, name=name + "_t")
            sv = S[:, 0:256].rearrange("p (m h) -> p m h", m=2).transpose([0, 2, 1])
            nc.vector.tensor_mul(P[:, :, :], R_t[:, :, lo : lo + 2], sv)
            n = work.tile([IPC, H], f32, tag=name + "n", name=name + "n_t")
            nc.vector.tensor_reduce(n[:], P[:, :, :], mybir.AxisListType.X, ALU.add)
            return n

        n0a = pmul2(S0a, 0, "P0a")
        n0b = pmul2(S0b, 2, "P0b")
        add0 = work.tile([IPC, H], f32, tag="add0", name="add0_t")
        nc.vector.tensor_add(add0[:], n0a[:], n0b[:])
        P1_t = work.tile([IPC, H], f32, tag="P1", name="P1_t")
        nc.vector.tensor_mul(P1_t[:], R_t[:, :, 4], S1[:, :])
        add1 = work.tile([IPC, H], f32, tag="add1", name="add1_t")
        nc.vector.tensor_add(add1[:], add0[:], P1_t[:])
        P2_t = work.tile([IPC, H], f32, tag="P2", name="P2_t")
        nc.vector.tensor_mul(P2_t[:], R_t[:, :, 5], S2[:, 0:128])
        nsum_t = work.tile([IPC, H], f32, tag="nsum", name="nsum_t")
        nc.vector.tensor_add(nsum_t[:], add1[:], P2_t[:])
        U_t = work.tile([IPC, H], bf16, tag="U", name="U_t")
        nc.vector.scalar_tensor_tensor(
            U_t[:], nsum_t[:], rs_t[:, 0:1], rden_t[:], ALU.mult, ALU.mult
        )

        # ---- transpose + epilogue ----
        UT_ps = small.tile([H, IPC], bf16, tag="sm3", name="UT_ps")
        nc.tensor.transpose(UT_ps[:], U_t[:], ident)
        UT_sb = work.tile([H, IPC], bf16, tag="UT", name="UT_sb")
        nc.scalar.activation(UT_sb[:], UT_ps[:], AF.Copy)

        c3 = small.tile([H, IPC], f32, tag="sm1", name="c3_ps")
        nc.tensor.matmul(c3[:], W32T, UT_sb[:], start=True, stop=True)
        t3 = work.tile([H, IPC], bf16, tag="t3", name="t3_sb")
        nc.scalar.activation(t3[:], c3[:], AF.Tanh, bias=b32)
        c4 = small.tile([H, IPC], f32, tag="sm2", name="c4_ps")
        nc.tensor.matmul(c4[:], W4T, t3[:], start=True, stop=True)
        dzT = work.tile([H, IPC], f32, tag="dzT", name="dzT_sb")
        nc.scalar.activation(dzT[:], c4[:], AF.Identity, bias=b4)
        nc.sync.dma_start(out[:, :], dzT[:])

    nc.finalize()
    return nc


def _get_nc():
    if "nc" not in _CACHE:
        _CACHE["nc"] = _build()
    return _CACHE["nc"]


def kernel(**inputs):
    global LAST_RESULTS
    from concourse.bass_utils import run_bass_kernel_spmd

    f32 = np.float32
    bf = ml_dtypes.bfloat16
    z = np.asarray(inputs["z"], dtype=f32)
    s_t = np.asarray(inputs["s_t"], dtype=f32)
    W1 = np.asarray(inputs["W1"], dtype=f32)
    b1 = np.asarray(inputs["b1"], dtype=f32)
    W2 = np.asarray(inputs["W2"], dtype=f32)
    b2 = np.asarray(inputs["b2"], dtype=f32)
    Wq = np.asarray(inputs["Wq"], dtype=f32)
    bq = np.asarray(inputs["bq"], dtype=f32)
    Wk = np.asarray(inputs["Wk"], dtype=f32)
    bk = np.asarray(inputs["bk"], dtype=f32)
    W3 = np.asarray(inputs["W3"], dtype=f32)
    b3 = np.asarray(inputs["b3"], dtype=f32)
    W4 = np.asarray(inputs["W4"], dtype=f32)
    b4 = np.asarray(inputs["b4"], dtype=f32)

    rt = f32(1.0 / np.sqrt(H))
    trb = lambda m: np.ascontiguousarray(m.T).astype(bf)
    xm, w = _nodes()

    pkF = np.zeros((128, 5 + 2 * M), f32)
    pkF[:, 0] = bq * rt
    pkF[:, 1] = bk
    pkF[:, 2] = W3 @ b2 + b3
    pkF[:, 4] = b4
    pkF[:, 5 : 5 + M] = xm
    pkF[:, 5 + M : 5 + 2 * M] = w

    pkC = np.zeros((128, 384), bf)
    pkC[:, 0:128] = trb(W3 @ W2)
    pkC[:, 128:256] = trb(W4)
    pkC[:, 256:384] = np.eye(IPC, dtype=f32).astype(bf)

    zC = {b: trb(z[b]) for b in range(B)}
    pk32b = {}
    for b in range(B):
        p = np.zeros((O, 896), bf)
        p[:, 0:128] = trb(Wk)
        p[:, 128:640] = trb(s_t[b])
        p[0, 640:768] = b1.astype(bf)
        p[0, 768:896] = 1.0
        pk32b[b] = p

    in_maps = []
    for c in range(NC):
        b, blk = divmod(c, CPB)
        i0 = blk * IPC
        zWa = np.zeros((128, 513), bf)
        zWa[:, 0:128] = trb(z[b, i0 : i0 + IPC])
        zWa[:, 128:256] = trb(Wq * rt)
        zWa[:, 256:384] = trb(W1[:, H:])
        zWa[:, 384:512] = trb(W1[:, :H])
        madd = np.zeros((128, N), f32)
        madd[np.arange(IPC), i0 + np.arange(IPC)] = -1e30
        in_maps.append(
            dict(
                zW=zWa,
                zC=zC[b],
                pkF=pkF,
                pk32=pk32b[b],
                Madd=madd.astype(bf),
                pkC=pkC,
            )
        )

    nc = _get_nc()
    res = run_bass_kernel_spmd(nc, in_maps, core_ids=list(range(NC)))
    LAST_RESULTS = res

    dz = np.empty((B, N, H), dtype=f32)
    for c in range(NC):
        b, blk = divmod(c, CPB)
        i0 = blk * IPC
        dz[b, i0 : i0 + IPC, :] = res.results[c]["out"].T
    return dz


# revision 3
# speedup vs baseline: 1.0790x; 1.0790x over previous
"""Trainium2 Bass kernel for the ODEFunc GNN message-passing module (final).

v4 + trace-driven fixes:
- real DMA transfer time gates the start: inputs split over 3 hardware
  queues (sync/scalar/gpsimd) in need-order
- per-group S PSUM tiles (P/num dependencies were whole-tile in v4, so the
  combination couldn't overlap the later S groups); the group tiles reuse
  the banks of kT/scT/yj which are dead by then
- S node-groups (2,2,1,1): the first group only needs tanh nodes 0-1, so
  PE and DVE overlap the scalar tanh chain instead of trailing it
- barycentric denominator back on DVE (gpsimd chain was slower and the
  static schedule head-of-line blocked the DVE tail behind it)
- kT bias-add on DVE, scalar runs only exp + tanh chain + epilogue
- bf16 U/identity/transpose
"""

import ml_dtypes
import numpy as np

B, N, H, O = 2, 512, 128, 32
NC = 8
CPB = NC // B
IPC = N // CPB
NCHUNK = N // 128

M = 6
XRANGE = 2.0

_CACHE = {}
LAST_RESULTS = None


def _nodes():
    m = np.arange(M)
    xm = (XRANGE * np.cos(np.pi * m / (M - 1))).astype(np.float32)
    w = (-1.0) ** m
    w[0] *= 0.5
    w[-1] *= 0.5
    return xm, w.astype(np.float32)


def _build():
    from contextlib import ExitStack

    import concourse.tile as tile
    from concourse import bacc, mybir

    f32 = mybir.dt.float32
    bf16 = mybir.dt.bfloat16
    AF = mybir.ActivationFunctionType
    ALU = mybir.AluOpType

    nc = bacc.Bacc(trn_type="TRN2")

    # zW  [128, 513] bf16: zTi | WqTs | W1jT | W1iT | onescol
    # zC  [128, 512] bf16: zT
    # pkF [128, 17]  f32 : bqs bk b2 b3 b4 | xnodes | wts
    # pk32 [32, 896] bf16: WkT | sT | (row0: b1row | onesrow)
    # maskT [128, 512] bf16 ; pkC [128, 512] bf16: W2T | W3T | W4T | ident
    zW = nc.dram_tensor("zW", [128, 384], bf16, kind="ExternalInput")
    zC = nc.dram_tensor("zC", [H, N], bf16, kind="ExternalInput")
    pkF = nc.dram_tensor("pkF", [128, 5 + 2 * M], f32, kind="ExternalInput")
    pk32 = nc.dram_tensor("pk32", [O, 896], bf16, kind="ExternalInput")
    pkC = nc.dram_tensor("pkC", [128, 512], bf16, kind="ExternalInput")
    out = nc.dram_tensor("out", [H, IPC], f32, kind="ExternalOutput")

    with tile.TileContext(nc) as tc, ExitStack() as ctx:
        const = ctx.enter_context(tc.tile_pool(name="const", bufs=1))
        work = ctx.enter_context(tc.tile_pool(name="work", bufs=2))
        pbig = ctx.enter_context(tc.tile_pool(name="pbig", bufs=1, space="PSUM"))
        small = ctx.enter_context(tc.tile_pool(name="small", bufs=1, space="PSUM"))

        zW_t = const.tile([128, 384], bf16, tag="zW", name="zW_sb")
        zC_t = const.tile([H, N], bf16, tag="zC", name="zC_sb")
        pkF_t = const.tile([128, 5 + 2 * M], f32, tag="pkF", name="pkF_sb")
        pk32_t = const.tile([O, 896], bf16, tag="pk32", name="pk32_sb")
        pkC_t = const.tile([128, 512], bf16, tag="pkC", name="pkC_sb")

        nc.sync.dma_start(zC_t[:], zC[:, :])
        nc.scalar.dma_start(pk32_t[:], pk32[:, :])
        nc.scalar.dma_start(zW_t[:], zW[:, :])
        nc.sync.dma_start(pkF_t[:], pkF[:, :])
        nc.gpsimd.dma_start(pkC_t[:], pkC[:, :])

        zTi = zC_t[:, 0:128]
        WqTs = zW_t[:, 0:128]
        W1jT = zW_t[:, 128:256]
        W1iT = zW_t[:, 256:384]
        zT = zC_t[:, :]
        WkT = pk32_t[:, 0:128]
        sT = pk32_t[:, 128:640]
        b1row = pk32_t[0:1, 640:768]
        onesrow = pk32_t[0:1, 768:896]
        bqs = pkF_t[:, 0:1]
        bk = pkF_t[:, 1:2]
        b32 = pkF_t[:, 2:3]
        b4 = pkF_t[:, 4:5]
        xnodes = pkF_t[:, 5 : 5 + M]
        wts = pkF_t[:, 5 + M : 5 + 2 * M]
        W32T = pkC_t[:, 0:128]
        W4T = pkC_t[:, 128:256]
        ident = pkC_t[:, 256:384]
        mident = pkC_t[:, 384:512]

        # ---- PE head ----
        kT_ps = pbig.tile([H, N], f32, tag="kT", name="kT_ps")
        nc.tensor.matmul(kT_ps[:], WkT, sT, start=True, stop=True)
        yj_ps = pbig.tile([128, NCHUNK, H], f32, tag="yj", name="yj_ps")
        for c in range(NCHUNK):
            nc.tensor.matmul(
                yj_ps[:, c, :],
                zT[:, 128 * c : 128 * (c + 1)],
                W1jT,
                start=True,
                stop=True,
            )
        scT_ps = pbig.tile([128, N], f32, tag="scT", name="scT_ps")
        nc.tensor.matmul(scT_ps[:, 0:128], ident, mident, start=True, stop=False)
        qs_ps = small.tile([H, IPC], f32, tag="sm1", name="qs_ps")
        nc.tensor.matmul(qs_ps[:], WqTs, zTi, start=True, stop=True)
        xi_ps = small.tile([IPC, H], f32, tag="sm2", name="xi_ps")
        nc.tensor.matmul(xi_ps[:], zTi, W1iT, start=True, stop=False)
        nc.tensor.matmul(xi_ps[:], onesrow, b1row, start=False, stop=True)

        # DVE: bias-adds for q and k
        qsT_t = work.tile([H, IPC], bf16, tag="qsT", name="qsT_sb")
        nc.vector.tensor_scalar(qsT_t[:], qs_ps[:], bqs, None, ALU.add)
        kT_t = work.tile([H, N], bf16, tag="kT", name="kT_sb")
        nc.vector.tensor_scalar(kT_t[:], kT_ps[:], bk, None, ALU.add)

        for c in range(NCHUNK):
            nc.tensor.matmul(
                scT_ps[:, 128 * c : 128 * (c + 1)],
                kT_t[:, 128 * c : 128 * (c + 1)],
                qsT_t[:],
                start=(c != 0),
                stop=True,
            )

        # scalar: tanh node chain with exp slotted after T1
        T_all = const.tile([128, NCHUNK, M * 128 + 1], bf16, tag="T_all", name="T_all")
        nc.gpsimd.memset(T_all[:, :, M * 128 : M * 128 + 1], 1.0)

        def t_act(m):
            nc.scalar.activation(
                T_all[:, :, 128 * m : 128 * (m + 1)],
                yj_ps[:, :, :],
                AF.Tanh,
                bias=xnodes[:, m : m + 1],
            )

        with tc.high_priority():
            t_act(0)
            t_act(1)
        eT_t = work.tile([128, N], bf16, tag="eT", name="eT_sb")
        nc.scalar.activation(eT_t[:], scT_ps[:], AF.Exp)
        for m in range(2, M):
            t_act(m)

        # ---- DVE: D, R0, mask, R, den, rden ----
        D_t = work.tile([IPC, H, M], f32, tag="D", name="D_t")
        xi_b = xi_ps[:, :].unsqueeze(2).broadcast_to((IPC, H, M))
        xn_b = xnodes.unsqueeze(1).broadcast_to((IPC, H, M))
        wt_b = wts.unsqueeze(1).broadcast_to((IPC, H, M))
        nc.vector.tensor_sub(D_t[:, :, :], xi_b, xn_b)
        R0_t = work.tile([IPC, H, M], f32, tag="R0", name="R0_t")
        nc.vector.reciprocal_approx_fast(R0_t[:, :, :], D_t[:, :, :])
        R_t = work.tile([IPC, H, M], f32, tag="R", name="R_t")
        nc.vector.tensor_mul(R_t[:, :, :], R0_t[:, :, :], wt_b)
        den_t = work.tile([IPC, H], f32, tag="den", name="den_t")
        nc.vector.tensor_reduce(den_t[:], R_t[:, :, :], mybir.AxisListType.X, ALU.add)
        rden_t = work.tile([IPC, H], f32, tag="rden", name="rden_t")
        nc.vector.reciprocal_approx_fast(rden_t[:], den_t[:])

        # ---- S matmul groups (2,2,1,1+ones) into recycled PSUM banks ----
        def s_group(pool, tag, tcol0, width, name):
            sp = pool.tile([IPC, width], f32, tag=tag, name=name)
            for c in range(NCHUNK):
                nc.tensor.matmul(
                    sp[:, :],
                    eT_t[:, 128 * c : 128 * (c + 1)],
                    T_all[:, c, tcol0 : tcol0 + width],
                    start=(c == 0),
                    stop=(c == NCHUNK - 1),
                )
            return sp

        Sones = s_group(small, "sm3", M * 128, 1, "Sones_ps")  # softmax denom
        rs_t = work.tile([IPC, 1], f32, tag="rs", name="rs_t")
        nc.vector.reciprocal_approx_fast(rs_t[:], Sones[:, 0:1])
        S0a = s_group(pbig, "kT", 0, 256, "S0a_ps")     # nodes 0-1
        S0b = s_group(pbig, "scT", 256, 256, "S0b_ps")  # nodes 2-3
        S1 = s_group(pbig, "yj", 512, 128, "S1_ps")     # node 4
        S2 = s_group(small, "sm1", 640, 128, "S2_ps")   # node 5

        # ---- DVE tail: per-group P/num, combine, U ----
        def pmul2(S, lo, name):
            P = work.tile([IPC, H, 2], f32, tag=name, name=name + "_t")
            sv = S[:, 0:256].rearrange("p (m h) -> p m h", m=2).transpose([0, 2, 1])
            nc.vector.tensor_mul(P[:, :, :], R_t[:, :, lo : lo + 2], sv)
            n = work.tile([IPC, H], f32, tag=name + "n", name=name + "n_t")
            nc.vector.tensor_reduce(n[:], P[:, :, :], mybir.AxisListType.X, ALU.add)
            return n

        n0a = pmul2(S0a, 0, "P0a")
        n0b = pmul2(S0b, 2, "P0b")
        add0 = work.tile([IPC, H], f32, tag="add0", name="add0_t")
        nc.vector.tensor_add(add0[:], n0a[:], n0b[:])
        P1_t = work.tile([IPC, H], f32, tag="P1", name="P1_t")
        nc.vector.tensor_mul(P1_t[:], R_t[:, :, 4], S1[:, :])
        add1 = work.tile([IPC, H], f32, tag="add1", name="add1_t")
        nc.vector.tensor_add(add1[:], add0[:], P1_t[:])
        P2_t = work.tile([IPC, H], f32, tag="P2", name="P2_t")
        nc.vector.tensor_mul(P2_t[:], R_t[:, :, 5], S2[:, 0:128])
        nsum_t = work.tile([IPC, H], f32, tag="nsum", name="nsum_t")
        nc.vector.tensor_add(nsum_t[:], add1[:], P2_t[:])
        U_t = work.tile([IPC, H], bf16, tag="U", name="U_t")
        nc.vector.scalar_tensor_tensor(
            U_t[:], nsum_t[:], rs_t[:, 0:1], rden_t[:], ALU.mult, ALU.mult
        )

        # ---- transpose + epilogue ----
        UT_ps = small.tile([H, IPC], bf16, tag="sm3", name="UT_ps")
        nc.tensor.transpose(UT_ps[:], U_t[:], ident)
        UT_sb = work.tile([H, IPC], bf16, tag="UT", name="UT_sb")
        nc.scalar.activation(UT_sb[:], UT_ps[:], AF.Copy)

        c3 = small.tile([H, IPC], f32, tag="sm1", name="c3_ps")
        nc.tensor.matmul(c3[:], W32T, UT_sb[:], start=True, stop=True)
        t3 = work.tile([H, IPC], bf16, tag="t3", name="t3_sb")
        nc.scalar.activation(t3[:], c3[:], AF.Tanh, bias=b32)
        c4 = small.tile([H, IPC], f32, tag="sm2", name="c4_ps")
        nc.tensor.matmul(c4[:], W4T, t3[:], start=True, stop=True)
        dzT = work.tile([H, IPC], f32, tag="dzT", name="dzT_sb")
        nc.scalar.activation(dzT[:], c4[:], AF.Identity, bias=b4)
        nc.sync.dma_start(out[:, :], dzT[:])

    nc.finalize()
    return nc


def _get_nc():
    if "nc" not in _CACHE:
        _CACHE["nc"] = _build()
    return _CACHE["nc"]


def kernel(**inputs):
    global LAST_RESULTS
    from concourse.bass_utils import run_bass_kernel_spmd

    f32 = np.float32
    bf = ml_dtypes.bfloat16
    z = np.asarray(inputs["z"], dtype=f32)
    s_t = np.asarray(inputs["s_t"], dtype=f32)
    W1 = np.asarray(inputs["W1"], dtype=f32)
    b1 = np.asarray(inputs["b1"], dtype=f32)
    W2 = np.asarray(inputs["W2"], dtype=f32)
    b2 = np.asarray(inputs["b2"], dtype=f32)
    Wq = np.asarray(inputs["Wq"], dtype=f32)
    bq = np.asarray(inputs["bq"], dtype=f32)
    Wk = np.asarray(inputs["Wk"], dtype=f32)
    bk = np.asarray(inputs["bk"], dtype=f32)
    W3 = np.asarray(inputs["W3"], dtype=f32)
    b3 = np.asarray(inputs["b3"], dtype=f32)
    W4 = np.asarray(inputs["W4"], dtype=f32)
    b4 = np.asarray(inputs["b4"], dtype=f32)

    rt = f32(1.0 / np.sqrt(H))
    trb = lambda m: np.ascontiguousarray(m.T).astype(bf)
    xm, w = _nodes()

    pkF = np.zeros((128, 5 + 2 * M), f32)
    pkF[:, 0] = bq * rt
    pkF[:, 1] = bk
    pkF[:, 2] = W3 @ b2 + b3
    pkF[:, 4] = b4
    pkF[:, 5 : 5 + M] = xm
    pkF[:, 5 + M : 5 + 2 * M] = w

    pkC = np.zeros((128, 512), bf)
    pkC[:, 0:128] = trb(W3 @ W2)
    pkC[:, 128:256] = trb(W4)
    pkC[:, 256:384] = np.eye(IPC, dtype=f32).astype(bf)
    pkC[:, 384:512] = (np.eye(IPC, dtype=f32) * np.float32(-1e30)).astype(bf)

    zTb = {b: np.ascontiguousarray(z[b].T) for b in range(B)}
    sTb = {b: np.ascontiguousarray(s_t[b].T) for b in range(B)}

    zWa = np.zeros((128, 384), bf)
    zWa[:, 0:128] = trb(Wq * rt)
    zWa[:, 128:256] = trb(W1[:, H:])
    zWa[:, 256:384] = trb(W1[:, :H])
    in_maps = []
    for c in range(NC):
        b, blk = divmod(c, CPB)
        i0 = blk * IPC
        p = np.zeros((O, 896), bf)
        p[:, 0:128] = trb(Wk)
        p[:, 128:640] = np.roll(sTb[b], -i0, axis=1).astype(bf)
        p[0, 640:768] = b1.astype(bf)
        p[0, 768:896] = 1.0
        in_maps.append(
            dict(
                zW=zWa,
                zC=np.roll(zTb[b], -i0, axis=1).astype(bf),
                pkF=pkF,
                pk32=p,
                pkC=pkC,
            )
        )

    nc = _get_nc()
    res = run_bass_kernel_spmd(nc, in_maps, core_ids=list(range(NC)))
    LAST_RESULTS = res

    dz = np.empty((B, N, H), dtype=f32)
    for c in range(NC):
        b, blk = divmod(c, CPB)
        i0 = blk * IPC
        dz[b, i0 : i0 + IPC, :] = res.results[c]["out"].T
    return dz


# revision 4
# speedup vs baseline: 1.1262x; 1.0438x over previous
"""Trainium2 Bass kernel for the ODEFunc GNN message-passing module (final).

v4 + trace-driven fixes:
- real DMA transfer time gates the start: inputs split over 3 hardware
  queues (sync/scalar/gpsimd) in need-order
- per-group S PSUM tiles (P/num dependencies were whole-tile in v4, so the
  combination couldn't overlap the later S groups); the group tiles reuse
  the banks of kT/scT/yj which are dead by then
- S node-groups (2,2,1,1): the first group only needs tanh nodes 0-1, so
  PE and DVE overlap the scalar tanh chain instead of trailing it
- barycentric denominator back on DVE (gpsimd chain was slower and the
  static schedule head-of-line blocked the DVE tail behind it)
- kT bias-add on DVE, scalar runs only exp + tanh chain + epilogue
- bf16 U/identity/transpose
"""

import ml_dtypes
import numpy as np

B, N, H, O = 2, 512, 128, 32
NC = 8
CPB = NC // B
IPC = N // CPB
NCHUNK = N // 128

M = 6
XRANGE = 2.0

_CACHE = {}
LAST_RESULTS = None


def _nodes():
    m = np.arange(M)
    xm = (XRANGE * np.cos(np.pi * m / (M - 1))).astype(np.float32)
    w = (-1.0) ** m
    w[0] *= 0.5
    w[-1] *= 0.5
    return xm, w.astype(np.float32)


def _build():
    from contextlib import ExitStack

    import concourse.tile as tile
    from concourse import bacc, mybir

    f32 = mybir.dt.float32
    bf16 = mybir.dt.bfloat16
    AF = mybir.ActivationFunctionType
    ALU = mybir.AluOpType

    nc = bacc.Bacc(trn_type="TRN2")

    # zW  [128, 513] bf16: zTi | WqTs | W1jT | W1iT | onescol
    # zC  [128, 512] bf16: zT
    # pkF [128, 17]  f32 : bqs bk b2 b3 b4 | xnodes | wts
    # pk32 [32, 896] bf16: WkT | sT | (row0: b1row | onesrow)
    # maskT [128, 512] bf16 ; pkC [128, 512] bf16: W2T | W3T | W4T | ident
    zW = nc.dram_tensor("zW", [128, 384], bf16, kind="ExternalInput")
    zC = nc.dram_tensor("zC", [H, N], bf16, kind="ExternalInput")
    pkF = nc.dram_tensor("pkF", [128, 5 + 2 * M], f32, kind="ExternalInput")
    pk32 = nc.dram_tensor("pk32", [O, 896], bf16, kind="ExternalInput")
    pkC = nc.dram_tensor("pkC", [128, 512], bf16, kind="ExternalInput")
    out = nc.dram_tensor("out", [H, IPC], f32, kind="ExternalOutput")

    with tile.TileContext(nc) as tc, ExitStack() as ctx:
        const = ctx.enter_context(tc.tile_pool(name="const", bufs=1))
        work = ctx.enter_context(tc.tile_pool(name="work", bufs=2))
        pbig = ctx.enter_context(tc.tile_pool(name="pbig", bufs=1, space="PSUM"))
        small = ctx.enter_context(tc.tile_pool(name="small", bufs=1, space="PSUM"))

        zW_t = const.tile([128, 384], bf16, tag="zW", name="zW_sb")
        zC_t = const.tile([H, N], bf16, tag="zC", name="zC_sb")
        pkF_t = const.tile([128, 5 + 2 * M], f32, tag="pkF", name="pkF_sb")
        pk32_t = const.tile([O, 896], bf16, tag="pk32", name="pk32_sb")
        pkC_t = const.tile([128, 512], bf16, tag="pkC", name="pkC_sb")

        nc.sync.dma_start(zC_t[:], zC[:, :])
        nc.scalar.dma_start(pk32_t[:], pk32[:, :])
        nc.scalar.dma_start(zW_t[:], zW[:, :])
        nc.sync.dma_start(pkF_t[:], pkF[:, :])
        nc.gpsimd.dma_start(pkC_t[:], pkC[:, :])

        zTi = zC_t[:, 0:128]
        WqTs = zW_t[:, 0:128]
        W1jT = zW_t[:, 128:256]
        W1iT = zW_t[:, 256:384]
        zT = zC_t[:, :]
        WkT = pk32_t[:, 0:128]
        sT = pk32_t[:, 128:640]
        b1row = pk32_t[0:1, 640:768]
        onesrow = pk32_t[0:1, 768:896]
        bqs = pkF_t[:, 0:1]
        bk = pkF_t[:, 1:2]
        b32 = pkF_t[:, 2:3]
        b4 = pkF_t[:, 4:5]
        xnodes = pkF_t[:, 5 : 5 + M]
        wts = pkF_t[:, 5 + M : 5 + 2 * M]
        W32T = pkC_t[:, 0:128]
        W4T = pkC_t[:, 128:256]
        ident = pkC_t[:, 256:384]
        mident = pkC_t[:, 384:512]

        # ---- PE head ----
        kT_ps = pbig.tile([H, N], f32, tag="kT", name="kT_ps")
        nc.tensor.matmul(kT_ps[:], WkT, sT, start=True, stop=True)
        yj_ps = pbig.tile([128, NCHUNK, H], f32, tag="yj", name="yj_ps")
        for c in range(NCHUNK):
            nc.tensor.matmul(
                yj_ps[:, c, :],
                zT[:, 128 * c : 128 * (c + 1)],
                W1jT,
                start=True,
                stop=True,
            )
        scT_ps = pbig.tile([128, N], f32, tag="scT", name="scT_ps")
        nc.tensor.matmul(scT_ps[:, 0:128], ident, mident, start=True, stop=False)
        qs_ps = small.tile([H, IPC], f32, tag="sm1", name="qs_ps")
        nc.tensor.matmul(qs_ps[:], WqTs, zTi, start=True, stop=True)
        xi_ps = small.tile([IPC, H], f32, tag="sm2", name="xi_ps")
        nc.tensor.matmul(xi_ps[:], zTi, W1iT, start=True, stop=False)
        nc.tensor.matmul(xi_ps[:], onesrow, b1row, start=False, stop=True)

        # DVE: bias-adds for q and k
        qsT_t = work.tile([H, IPC], bf16, tag="qsT", name="qsT_sb")
        nc.vector.tensor_scalar(qsT_t[:], qs_ps[:], bqs, None, ALU.add)
        kT_t = work.tile([H, N], bf16, tag="kT", name="kT_sb")
        nc.vector.tensor_scalar(kT_t[:, 0:256], kT_ps[:, 0:256], bk, None, ALU.add)
        nc.scalar.activation(kT_t[:, 256:512], kT_ps[:, 256:512], AF.Identity, bias=bk)

        for c in range(NCHUNK):
            nc.tensor.matmul(
                scT_ps[:, 128 * c : 128 * (c + 1)],
                kT_t[:, 128 * c : 128 * (c + 1)],
                qsT_t[:],
                start=(c != 0),
                stop=True,
            )

        # scalar: tanh node chain with exp slotted after T1
        T_all = const.tile([128, NCHUNK, M * 128 + 1], bf16, tag="T_all", name="T_all")
        nc.gpsimd.memset(T_all[:, :, M * 128 : M * 128 + 1], 1.0)

        def t_act(m):
            nc.scalar.activation(
                T_all[:, :, 128 * m : 128 * (m + 1)],
                yj_ps[:, :, :],
                AF.Tanh,
                bias=xnodes[:, m : m + 1],
            )

        with tc.high_priority():
            t_act(0)
            t_act(1)
        eT_t = work.tile([128, N], bf16, tag="eT", name="eT_sb")
        nc.scalar.activation(eT_t[:], scT_ps[:], AF.Exp)
        for m in range(2, M):
            t_act(m)

        # ---- DVE: D, R0, mask, R, den, rden ----
        D_t = work.tile([IPC, H, M], f32, tag="D", name="D_t")
        xi_b = xi_ps[:, :].unsqueeze(2).broadcast_to((IPC, H, M))
        xn_b = xnodes.unsqueeze(1).broadcast_to((IPC, H, M))
        wt_b = wts.unsqueeze(1).broadcast_to((IPC, H, M))
        nc.vector.tensor_sub(D_t[:, :, :], xi_b, xn_b)
        R0_t = work.tile([IPC, H, M], f32, tag="R0", name="R0_t")
        nc.vector.reciprocal_approx_fast(R0_t[:, :, :], D_t[:, :, :])
        R_t = work.tile([IPC, H, M], f32, tag="R", name="R_t")
        nc.vector.tensor_mul(R_t[:, :, :], R0_t[:, :, :], wt_b)
        den_t = work.tile([IPC, H], f32, tag="den", name="den_t")
        nc.vector.tensor_reduce(den_t[:], R_t[:, :, :], mybir.AxisListType.X, ALU.add)
        rden_t = work.tile([IPC, H], f32, tag="rden", name="rden_t")
        nc.vector.reciprocal_approx_fast(rden_t[:], den_t[:])

        # ---- S matmul groups (2,2,1,1+ones) into recycled PSUM banks ----
        def s_group(pool, tag, tcol0, width, name):
            sp = pool.tile([IPC, width], f32, tag=tag, name=name)
            for c in range(NCHUNK):
                nc.tensor.matmul(
                    sp[:, :],
                    eT_t[:, 128 * c : 128 * (c + 1)],
                    T_all[:, c, tcol0 : tcol0 + width],
                    start=(c == 0),
                    stop=(c == NCHUNK - 1),
                )
            return sp

        Sones = s_group(small, "sm3", M * 128, 1, "Sones_ps")  # softmax denom
        rs_t = work.tile([IPC, 1], f32, tag="rs", name="rs_t")
        nc.vector.reciprocal_approx_fast(rs_t[:], Sones[:, 0:1])
        S0a = s_group(pbig, "kT", 0, 256, "S0a_ps")     # nodes 0-1
        S0b = s_group(pbig, "scT", 256, 256, "S0b_ps")  # nodes 2-3
        S1 = s_group(pbig, "yj", 512, 128, "S1_ps")     # node 4
        S2 = s_group(small, "sm1", 640, 128, "S2_ps")   # node 5

        # ---- DVE tail: per-group P/num, combine, U ----
        def pmul2(S, lo, name):
            P = work.tile([IPC, H, 2], f32, tag=name, name=name + "_t")
            sv = S[:, 0:256].rearrange("p (m h) -> p m h", m=2).transpose([0, 2, 1])
            nc.vector.tensor_mul(P[:, :, :], R_t[:, :, lo : lo + 2], sv)
            n = work.tile([IPC, H], f32, tag=name + "n", name=name + "n_t")
            nc.vector.tensor_reduce(n[:], P[:, :, :], mybir.AxisListType.X, ALU.add)
            return n

        n0a = pmul2(S0a, 0, "P0a")
        n0b = pmul2(S0b, 2, "P0b")
        add0 = work.tile([IPC, H], f32, tag="add0", name="add0_t")
        nc.vector.tensor_add(add0[:], n0a[:], n0b[:])
        P1_t = work.tile([IPC, H], f32, tag="P1", name="P1_t")
        nc.vector.tensor_mul(P1_t[:], R_t[:, :, 4], S1[:, :])
        add1 = work.tile([IPC, H], f32, tag="add1", name="add1_t")
        nc.vector.tensor_add(add1[:], add0[:], P1_t[:])
        P2_t = work.tile([IPC, H], f32, tag="P2", name="P2_t")
        nc.vector.tensor_mul(P2_t[:], R_t[:, :, 5], S2[:, 0:128])
        nsum_t = work.tile([IPC, H], f32, tag="nsum", name="nsum_t")
        nc.vector.tensor_add(nsum_t[:], add1[:], P2_t[:])
        U_t = work.tile([IPC, H], bf16, tag="U", name="U_t")
        nc.vector.scalar_tensor_tensor(
            U_t[:], nsum_t[:], rs_t[:, 0:1], rden_t[:], ALU.mult, ALU.mult
        )

        # ---- transpose + epilogue ----
        UT_ps = small.tile([H, IPC], bf16, tag="sm3", name="UT_ps")
        nc.tensor.transpose(UT_ps[:], U_t[:], ident)
        UT_sb = work.tile([H, IPC], bf16, tag="UT", name="UT_sb")
        nc.scalar.activation(UT_sb[:], UT_ps[:], AF.Copy)

        c3 = small.tile([H, IPC], f32, tag="sm1", name="c3_ps")
        nc.tensor.matmul(c3[:], W32T, UT_sb[:], start=True, stop=True)
        t3 = work.tile([H, IPC], bf16, tag="t3", name="t3_sb")
        nc.scalar.activation(t3[:], c3[:], AF.Tanh, bias=b32)
        c4 = small.tile([H, IPC], f32, tag="sm2", name="c4_ps")
        nc.tensor.matmul(c4[:], W4T, t3[:], start=True, stop=True)
        dzT = work.tile([H, IPC], f32, tag="dzT", name="dzT_sb")
        nc.scalar.activation(dzT[:], c4[:], AF.Identity, bias=b4)
        nc.sync.dma_start(out[:, :], dzT[:])

    nc.finalize()
    return nc


def _get_nc():
    if "nc" not in _CACHE:
        _CACHE["nc"] = _build()
    return _CACHE["nc"]


def kernel(**inputs):
    global LAST_RESULTS
    from concourse.bass_utils import run_bass_kernel_spmd

    f32 = np.float32
    bf = ml_dtypes.bfloat16
    z = np.asarray(inputs["z"], dtype=f32)
    s_t = np.asarray(inputs["s_t"], dtype=f32)
    W1 = np.asarray(inputs["W1"], dtype=f32)
    b1 = np.asarray(inputs["b1"], dtype=f32)
    W2 = np.asarray(inputs["W2"], dtype=f32)
    b2 = np.asarray(inputs["b2"], dtype=f32)
    Wq = np.asarray(inputs["Wq"], dtype=f32)
    bq = np.asarray(inputs["bq"], dtype=f32)
    Wk = np.asarray(inputs["Wk"], dtype=f32)
    bk = np.asarray(inputs["bk"], dtype=f32)
    W3 = np.asarray(inputs["W3"], dtype=f32)
    b3 = np.asarray(inputs["b3"], dtype=f32)
    W4 = np.asarray(inputs["W4"], dtype=f32)
    b4 = np.asarray(inputs["b4"], dtype=f32)

    rt = f32(1.0 / np.sqrt(H))
    trb = lambda m: np.ascontiguousarray(m.T).astype(bf)
    xm, w = _nodes()

    pkF = np.zeros((128, 5 + 2 * M), f32)
    pkF[:, 0] = bq * rt
    pkF[:, 1] = bk
    pkF[:, 2] = W3 @ b2 + b3
    pkF[:, 4] = b4
    pkF[:, 5 : 5 + M] = xm
    pkF[:, 5 + M : 5 + 2 * M] = w

    pkC = np.zeros((128, 512), bf)
    pkC[:, 0:128] = trb(W3 @ W2)
    pkC[:, 128:256] = trb(W4)
    pkC[:, 256:384] = np.eye(IPC, dtype=f32).astype(bf)
    pkC[:, 384:512] = (np.eye(IPC, dtype=f32) * np.float32(-1e30)).astype(bf)

    zTb = {b: np.ascontiguousarray(z[b].T) for b in range(B)}
    sTb = {b: np.ascontiguousarray(s_t[b].T) for b in range(B)}

    zWa = np.zeros((128, 384), bf)
    zWa[:, 0:128] = trb(Wq * rt)
    zWa[:, 128:256] = trb(W1[:, H:])
    zWa[:, 256:384] = trb(W1[:, :H])
    in_maps = []
    for c in range(NC):
        b, blk = divmod(c, CPB)
        i0 = blk * IPC
        p = np.zeros((O, 896), bf)
        p[:, 0:128] = trb(Wk)
        p[:, 128:640] = np.roll(sTb[b], -i0, axis=1).astype(bf)
        p[0, 640:768] = b1.astype(bf)
        p[0, 768:896] = 1.0
        in_maps.append(
            dict(
                zW=zWa,
                zC=np.roll(zTb[b], -i0, axis=1).astype(bf),
                pkF=pkF,
                pk32=p,
                pkC=pkC,
            )
        )

    nc = _get_nc()
    res = run_bass_kernel_spmd(nc, in_maps, core_ids=list(range(NC)))
    LAST_RESULTS = res

    dz = np.empty((B, N, H), dtype=f32)
    for c in range(NC):
        b, blk = divmod(c, CPB)
        i0 = blk * IPC
        dz[b, i0 : i0 + IPC, :] = res.results[c]["out"].T
    return dz
